# revision 4
# baseline (speedup 1.0000x reference)
"""BitLinear forward on 8 Trainium2 NeuronCores (v2).

Computes y = x @ (unpack_bits(bp).reshape(OUT, IN) * scale).T for
x[64, 4096] fp32, bp[OUT*IN/8] int32 (8 sign bits per int, MSB-first),
scale[OUT, 1] fp32, OUT=11008, IN=4096.

Strategy (column-parallel / output-feature sharded, no collectives):
  * Each core owns 1376 output rows, padded to 1408 = 11 * 128.
  * Host re-lays bp as uint16 bpT[g, o] (g = in-feature group of 8) and
    x as 2 * x.T with rows permuted to (chunk c, bit j) order, bf16.
  * Warm-up matmuls on junk data run during the input DMA so the PE HAM
    clock-gate is released (2.4 GHz) before the real matmuls start.
  * Unpack: j0 (MSB) via ScalarE Sign (+-1 plane, 1x rows); j7 (LSB) as
    +-1 bf16 bit patterns via one DVE op (v << 15) ^ 0xBF80; j1..j6 as
    DVE (v>>s)&1 (uint16) plus a cast copy distributed across DVE,
    ScalarE, and GpSimd (0/1 planes, 2x rows, -sum(x) bias).
  * PE accumulates bf16 matmuls psum[t, o] += xt.T @ plane with column
    tiling by j-parity: even-j planes accumulate in rows 0:64 of pe
    PSUM banks, odd-j in rows 64:128 of po banks, so adjacent-j matmuls
    run concurrently in the PE array. Sweeps are c-major so work starts
    as soon as each 128-group chunk of bpt lands.
  * Epilogue per output chunk (og-major on c3 so each og's epilogue
    overlaps remaining matmuls): PSUM->SBUF copies add the per-token
    -sum(x) correction as a bias; an fp16 matmul against a stacked
    [I; I] matrix transposes y to [o, t] and sums the parity halves;
    DVE applies the per-output-row scale; output DMA is chunked.
  * Host concatenates core outputs and transposes back to [64, OUT].
"""

import numpy as np
import ml_dtypes

OUT, IN, TOKENS = 11008, 4096, 64
NCORES = 8
P = 128
G = IN // 8              # 512 in-feature groups (bytes per output row)
OPC = 1408               # padded output rows per core (11 * 128)
W4 = 4 * OPC             # all four g-chunks side by side
OUT_PAD = NCORES * OPC   # 11264
KCH = OPC // P           # 11 output chunks of 128 rows per core
OG_SIZES = [512, 512, 384]  # psum free-dim chunking of 1408
OG_STARTS = [0, 512, 1024]
OG_KS = [range(0, 4), range(4, 8), range(8, 11)]  # 128-chunks per og
N_WARM = 10              # HAM warm-up matmuls

# cast-engine per (j, chunk) for the middle planes j=1..6.
# chunks: 0..3 = c0..c3. d=DVE, a=ACT (ScalarE), g=GpSimd.
CAST_ENG = {
    1: ["d", "g", "g", "g"],
    2: ["g", "a", "d", "a"],
    3: ["d", "g", "a", "g"],
    4: ["g", "d", "g", "d"],
    5: ["d", "g", "d", "g"],
    6: ["g", "a", "g", "d"],
}

_CACHE = {}


def _build_bass():
    """Build + compile the per-core Bass kernel (identical on all cores)."""
    from contextlib import ExitStack

    import concourse.bass as bass
    import concourse.mybir as mybir
    import concourse.tile as tile
    from concourse import bacc
    from concourse.masks import make_identity

    nc = bacc.Bacc("TRN2", target_bir_lowering=False, debug=False)

    bpt = nc.dram_tensor("bpt", (G, OPC), mybir.dt.uint16, kind="ExternalInput")
    xt = nc.dram_tensor("xt", (P, 32 * TOKENS), mybir.dt.bfloat16, kind="ExternalInput")
    negsx = nc.dram_tensor("negsx", (P, 1), mybir.dt.float32, kind="ExternalInput")
    scale_t = nc.dram_tensor("scale_t", (P, KCH), mybir.dt.float32, kind="ExternalInput")
    yt = nc.dram_tensor("yt", (P, KCH * TOKENS), mybir.dt.float32, kind="ExternalOutput")

    with tile.TileContext(nc) as tc, ExitStack() as ctx:
        consts = ctx.enter_context(tc.tile_pool(name="consts", bufs=1))
        plane_pool = ctx.enter_context(tc.tile_pool(name="planes", bufs=12))
        upool = ctx.enter_context(tc.tile_pool(name="uplanes", bufs=8))
        out_pool = ctx.enter_context(tc.tile_pool(name="outs", bufs=1))
        psum_y = ctx.enter_context(tc.tile_pool(name="psum_y", bufs=1, space="PSUM"))
        psum_t = ctx.enter_context(tc.tile_pool(name="psum_t", bufs=2, space="PSUM"))

        # --- warm-up data (junk) + PSUM tiles ---
        pe_tiles = [
            psum_y.tile([P, w], mybir.dt.float32, name=f"psum_e{og}")
            for og, w in enumerate(OG_SIZES)
        ]
        po_tiles = [
            psum_y.tile([P, w], mybir.dt.float32, name=f"psum_o{og}")
            for og, w in enumerate(OG_SIZES)
        ]

        wl = consts.tile([P, TOKENS], mybir.dt.bfloat16, name="warm_l")
        wr = consts.tile([P, 512], mybir.dt.bfloat16, name="warm_r")
        nc.vector.memset(wl[:], 0.0)
        nc.vector.memset(wr[:], 0.0)
        for i in range(N_WARM):
            nc.tensor.matmul(
                pe_tiles[0][:TOKENS, :], wl[:], wr[:],
                start=True, stop=True, tile_position=(0, 0),
            )

        # --- inputs to SBUF ---
        # bpt chunks on the sync HWDGE ring (c0 split so the og0 column
        # range lands first); xt split on the scalar ring.
        bpt_all = consts.tile([P, W4], mybir.dt.uint16, name="bpt_all")
        xt_s = consts.tile([P, 32 * TOKENS], mybir.dt.bfloat16, name="xt_s")

        nc.sync.dma_start(bpt_all[:, :512], bpt[0:P, :512])
        nc.scalar.dma_start(xt_s[:, :8 * TOKENS], xt[:, :8 * TOKENS])
        nc.sync.dma_start(bpt_all[:, 512:OPC], bpt[0:P, 512:])
        nc.scalar.dma_start(xt_s[:, 8 * TOKENS:], xt[:, 8 * TOKENS:])
        for c in range(1, 4):
            nc.sync.dma_start(bpt_all[:, c * OPC:(c + 1) * OPC],
                              bpt[c * P:(c + 1) * P, :])

        scale_s = consts.tile([P, KCH], mybir.dt.float32, name="scale_s")
        nc.scalar.dma_start(scale_s[:], scale_t[:, :])

        # per-partition bias: rows 0:64 = -sum(x) per token, rows 64:128 = 0
        negsx_s = consts.tile([P, 1], mybir.dt.float32, name="negsx_s")
        nc.scalar.dma_start(negsx_s[:], negsx[:, :])

        # bias constant for the ScalarE Sign-plane (j=0): sign(v - 127.5)
        bias128 = consts.tile([P, 1], mybir.dt.float32, name="bias128")
        nc.vector.memset(bias128[:], -127.5)

        # M2: [128, 64] = [identity_64; identity_64] — the epilogue matmul
        # ybuf_chunk.T @ M2 transposes y AND sums the parity halves.
        m2 = consts.tile([P, TOKENS], mybir.dt.float16, name="m2")
        make_identity(nc, m2[:TOKENS, :])
        make_identity(nc, m2[TOKENS:, :])

        ybuf = out_pool.tile([P, OPC], mybir.dt.float16, name="ybuf")
        out_s = out_pool.tile([P, KCH * TOKENS], mybir.dt.float32, name="out_s")

        def plane_mm(plane_ap, j, c, og, col0):
            """col0: column in plane_ap where chunk c's data starts."""
            m = c * 8 + j
            half = j % 2
            base = half * TOKENS
            tiles = po_tiles if half else pe_tiles
            w = OG_SIZES[og]
            s0 = col0 + OG_STARTS[og]
            nc.tensor.matmul(
                tiles[og][base:base + TOKENS, :],
                xt_s[:, m * TOKENS:(m + 1) * TOKENS],
                plane_ap[:, s0:s0 + w],
                start=(c == 0 and j == half),
                stop=(c == 3 and j == 6 + half),
                tile_position=(0, base),
            )

        def epilogue_og(og):
            w = OG_SIZES[og]
            s0, s1 = OG_STARTS[og], OG_STARTS[og] + w
            # PSUM -> SBUF with -sum(x)/0 per-row bias; even half on DVE,
            # odd half on ScalarE so they run in parallel
            nc.vector.tensor_scalar(
                ybuf[:TOKENS, s0:s1], pe_tiles[og][:TOKENS, :],
                negsx_s[:TOKENS, :], None, mybir.AluOpType.add,
            )
            nc.scalar.activation(
                ybuf[TOKENS:, s0:s1], po_tiles[og][TOKENS:, :],
                mybir.ActivationFunctionType.Identity,
                bias=negsx_s[TOKENS:, :], scale=1.0,
            )
            ks = list(OG_KS[og])
            pairs = [ks[i:i + 2] for i in range(0, len(ks), 2)]
            for pair in pairs:
                # [128,128].T @ [128,64] per chunk: transpose to [o, t] and
                # sum the parity halves via stacked identities; two chunks
                # share one PSUM tile so one DVE op scales both
                pt = psum_t.tile([P, 2 * TOKENS], mybir.dt.float32,
                                 name="psum_t")
                for i, k in enumerate(pair):
                    nc.tensor.matmul(
                        pt[:, i * TOKENS:(i + 1) * TOKENS],
                        ybuf[:, k * P:(k + 1) * P], m2[:, :],
                        start=True, stop=True,
                    )
                k0, n = pair[0], len(pair)
                # per-output-row scale while copying PSUM -> SBUF
                nc.vector.tensor_tensor(
                    out_s[:, k0 * TOKENS:(k0 + n) * TOKENS].rearrange(
                        "p (n t) -> p n t", n=n),
                    pt[:, :n * TOKENS].rearrange("p (n t) -> p n t", n=n),
                    scale_s[:, k0:k0 + n, None].to_broadcast((P, n, TOKENS)),
                    mybir.AluOpType.mult,
                )

        def extract_chunk(j, c, lo, hi, plane_t, u_t, cast_eng):
            """Extract plane j columns [lo:hi) of chunk c into plane_t."""
            col0 = c * OPC
            src = bpt_all[:, col0 + lo:col0 + hi]
            if j == 0:
                # MSB as a +-1 plane on ScalarE: sign(v - 127.5)
                nc.scalar.activation(
                    plane_t[:, lo:hi], src,
                    mybir.ActivationFunctionType.Sign,
                    bias=bias128[:, :], scale=1.0,
                )
                return
            if j == 7:
                # LSB plane as +-1 bf16 bit patterns in one uint16 op:
                # (v << 15) ^ 0xBF80
                nc.vector.tensor_scalar(
                    u_t[:, lo:hi], src, 15, 0xBF80,
                    mybir.AluOpType.logical_shift_left,
                    mybir.AluOpType.bitwise_xor,
                )
                return
            # middle bits: (v >> s) & 1 in uint16, then a casting copy
            nc.vector.tensor_scalar(
                u_t[:, lo:hi], src, 7 - j, 1,
                mybir.AluOpType.logical_shift_right,
                mybir.AluOpType.bitwise_and,
            )
            if cast_eng == "d":
                nc.vector.tensor_copy(plane_t[:, lo:hi], u_t[:, lo:hi])
            elif cast_eng == "a":
                nc.scalar.copy(plane_t[:, lo:hi], u_t[:, lo:hi])
            else:
                nc.gpsimd.tensor_copy(plane_t[:, lo:hi], u_t[:, lo:hi])

        # --- unpack + matmul rounds, c-major ---
        # planes[j] = (plane_tile or None-for-u-tile, u_tile) for current c
        for c in range(4):
            planes = {}
            for j in range(8):
                if j == 0:
                    pt_ = plane_pool.tile([P, OPC], mybir.dt.bfloat16,
                                          name="pl")
                    ut_ = None
                elif j == 7:
                    ut_ = upool.tile([P, OPC], mybir.dt.uint16, name="u")
                    pt_ = None
                else:
                    pt_ = plane_pool.tile([P, OPC], mybir.dt.bfloat16,
                                          name="pl")
                    ut_ = upool.tile([P, OPC], mybir.dt.uint16, name="u")
                cast_eng = CAST_ENG.get(j, [None] * 4)[c]
                if c == 0 and j <= 1:
                    # split so og0's column range unlocks first
                    extract_chunk(j, c, 0, 512, pt_, ut_, cast_eng)
                    extract_chunk(j, c, 512, OPC, pt_, ut_, cast_eng)
                else:
                    extract_chunk(j, c, 0, OPC, pt_, ut_, cast_eng)
                planes[j] = (pt_, ut_)

            if c < 3:
                for j in range(8):
                    pt_, ut_ = planes[j]
                    ap = ut_[:].bitcast(mybir.dt.bfloat16) if j == 7 else pt_
                    for og in range(3):
                        plane_mm(ap, j, c, og, 0)
            else:
                for og in range(3):
                    for j in range(8):
                        pt_, ut_ = planes[j]
                        ap = ut_[:].bitcast(mybir.dt.bfloat16) if j == 7 else pt_
                        plane_mm(ap, j, c, og, 0)
                    epilogue_og(og)

        # output DMA chunked per og so early chunks overlap the remaining
        # epilogue work
        nc.sync.dma_start(yt[:, :4 * TOKENS], out_s[:, :4 * TOKENS])
        nc.sync.dma_start(yt[:, 4 * TOKENS:8 * TOKENS],
                          out_s[:, 4 * TOKENS:8 * TOKENS])
        nc.sync.dma_start(yt[:, 8 * TOKENS:], out_s[:, 8 * TOKENS:])

    nc.compile()
    return nc


def _prep_inputs(x, bp, scale):
    """Host-side re-layout of the full inputs into 8 per-core input maps."""
    x = np.asarray(x, dtype=np.float32)
    bp = np.asarray(bp)
    scale = np.asarray(scale, dtype=np.float32)

    # packed bytes, transposed to [g, o_padded]
    bpm = np.zeros((G, OUT_PAD), dtype=np.uint16)
    bpm[:, :OUT] = bp.astype(np.uint16).reshape(OUT, G).T

    # xt = 2 * x.T with rows permuted to (c, j, g%128) order, split into
    # 128-row blocks laid out along the free dim: xt_dev[p, m*64 + t]
    # with m = c*8 + j. The j=0 / j=7 planes are +-1-valued, so those
    # rows use 1*x and those features are excluded from -sum(x).
    xT2 = (2.0 * x).T.astype(np.float32)            # [IN, TOKENS]
    xr = xT2.reshape(G, 8, TOKENS).reshape(4, P, 8, TOKENS)
    xr = xr.transpose(0, 2, 1, 3).copy()            # [c, j, p, t]
    xr[:, 0] *= 0.5                                 # j=0 rows: 1*x
    xr[:, 7] *= 0.5                                 # j=7 rows: 1*x
    xt_dev = np.ascontiguousarray(
        xr.reshape(32, P, TOKENS).transpose(1, 0, 2).reshape(P, 32 * TOKENS)
    ).astype(ml_dtypes.bfloat16)

    negsx = np.zeros((P, 1), dtype=np.float32)
    x64 = x.astype(np.float64)
    negsx[:TOKENS, 0] = (
        -x64.sum(axis=1) + x64[:, 0::8].sum(axis=1) + x64[:, 7::8].sum(axis=1)
    ).astype(np.float32)

    scale_pad = np.zeros(OUT_PAD, dtype=np.float32)
    scale_pad[:OUT] = scale.reshape(-1)

    in_maps = []
    for cid in range(NCORES):
        sl = slice(cid * OPC, (cid + 1) * OPC)
        in_maps.append({
            "bpt": np.ascontiguousarray(bpm[:, sl]),
            "xt": xt_dev,
            "negsx": negsx,
            "scale_t": np.ascontiguousarray(
                scale_pad[sl].reshape(KCH, P).T),
        })
    return in_maps


def _assemble(results):
    """results: per-core {'yt': [128, 11*64]} -> full [64, OUT] fp32."""
    parts = []
    for cid in range(NCORES):
        a = np.asarray(results[cid]["yt"], dtype=np.float32)
        parts.append(a.reshape(P, KCH, TOKENS).transpose(1, 0, 2).reshape(OPC, TOKENS))
    full = np.concatenate(parts, axis=0)[:OUT]      # [OUT, TOKENS]
    return np.ascontiguousarray(full.T)             # [TOKENS, OUT]


def kernel(x, bp, scale, _trace=False):
    from concourse import bass_utils

    if "nc" not in _CACHE:
        _CACHE["nc"] = _build_bass()
    nc = _CACHE["nc"]

    in_maps = _prep_inputs(x, bp, scale)
    res = bass_utils.run_bass_kernel_spmd(
        nc, in_maps, core_ids=list(range(NCORES)), trace=_trace,
    )
    _CACHE["last_result"] = res
    return _assemble(res.results)


# revision 6
# speedup vs baseline: 2.7302x; 2.7302x over previous
"""BitLinear forward on 8 Trainium2 NeuronCores (v3: fp8 byte-pair planes).

Computes y = x @ (unpack_bits(bp).reshape(OUT, IN) * scale).T for
x[64, 4096] fp32, bp[OUT*IN/8] int32 (8 sign bits per int, MSB-first),
scale[OUT, 1] fp32, OUT=11008, IN=4096.

Strategy (column-parallel / output-feature sharded, no collectives):
  * Each core owns 1376 output rows, padded to 1408 = 11 * 128.
  * Host pairs adjacent output features' packed bytes into uint16 words
    bpt2[g, o/2] = byte(g, 2o+1)<<8 | byte(g, 2o), so one DVE uint16 op
    produces TWO fp8 plane elements: the +-1 weights for bit j are
    synthesized as fp8e4 bit patterns 0x38/0xB8 via
    ((w << j) & 0x8080) ^ 0xB8B8 (j=0 fuses to one instruction).
    No casts, no 0/1 bias correction - planes are the real +-1 weights.
  * The PE multiplies bf16 x-tiles (stationary) against the fp8 planes
    (moving): mixed-dtype matmul, verified exact on hardware.
  * Warm-up matmuls on junk data run into unused PSUM rows during the
    input DMA so the PE HAM clock-gate is released (2.4 GHz) early.
  * Column tiling by j-parity: even-j planes accumulate in rows 0:64 of
    pe PSUM banks, odd-j in rows 64:128 of po banks, so adjacent-j
    matmuls run concurrently in the PE array. Sweeps are c-major so
    work starts as each 128-group chunk of bpt2 lands.
  * Epilogue per output chunk (og-major on the last c so each og's
    epilogue overlaps remaining matmuls): PSUM->SBUF fp16 copies split
    across DVE/ScalarE; an fp16 matmul against a stacked [I; I] matrix
    transposes y to [o, t] and sums the parity halves; DVE applies the
    per-output-row scale; output DMA is chunked.
  * Host concatenates core outputs and transposes back to [64, OUT].
"""

import numpy as np
import ml_dtypes

OUT, IN, TOKENS = 11008, 4096, 64
NCORES = 8
P = 128
G = IN // 8              # 512 in-feature groups (bytes per output row)
OPC = 1408               # padded output rows per core (11 * 128)
NW = OPC // 2            # 704 uint16 byte-pair words per row-chunk
OUT_PAD = NCORES * OPC   # 11264
KCH = OPC // P           # 11 output chunks of 128 rows per core
OG_SIZES = [512, 512, 384]  # psum free-dim chunking of 1408 (fp8 elems)
OG_STARTS = [0, 512, 1024]
OG_KS = [range(0, 4), range(4, 8), range(8, 11)]  # 128-chunks per og
N_WARM = 7               # HAM warm-up matmuls

_CACHE = {}


def _build_bass():
    """Build + compile the per-core Bass kernel (identical on all cores)."""
    from contextlib import ExitStack

    import concourse.bass as bass
    import concourse.mybir as mybir
    import concourse.tile as tile
    from concourse import bacc
    from concourse.masks import make_identity

    nc = bacc.Bacc("TRN2", target_bir_lowering=False, debug=False)

    bpt = nc.dram_tensor("bpt", (G, NW), mybir.dt.uint16, kind="ExternalInput")
    xt = nc.dram_tensor("xt", (P, 32 * TOKENS), mybir.dt.bfloat16, kind="ExternalInput")
    scale_t = nc.dram_tensor("scale_t", (P, KCH), mybir.dt.float32, kind="ExternalInput")
    yt = nc.dram_tensor("yt", (P, KCH * TOKENS), mybir.dt.float32, kind="ExternalOutput")

    with tile.TileContext(nc) as tc, ExitStack() as ctx:
        consts = ctx.enter_context(tc.tile_pool(name="consts", bufs=1))
        plane_pool = ctx.enter_context(tc.tile_pool(name="planes", bufs=12))
        upool = ctx.enter_context(tc.tile_pool(name="uplanes", bufs=6))
        out_pool = ctx.enter_context(tc.tile_pool(name="outs", bufs=1))
        psum_y = ctx.enter_context(tc.tile_pool(name="psum_y", bufs=1, space="PSUM"))
        psum_t = ctx.enter_context(tc.tile_pool(name="psum_t", bufs=2, space="PSUM"))

        pe_tiles = [
            psum_y.tile([P, w], mybir.dt.float32, name=f"psum_e{og}")
            for og, w in enumerate(OG_SIZES)
        ]
        po_tiles = [
            psum_y.tile([P, w], mybir.dt.float32, name=f"psum_o{og}")
            for og, w in enumerate(OG_SIZES)
        ]

        # --- HAM warm-up: junk matmuls into the never-used upper rows of
        # pe_tiles[0] (real pe matmuls only touch rows 0:64) ---
        wl = consts.tile([P, TOKENS], mybir.dt.bfloat16, name="warm_l")
        wr = consts.tile([P, 512], mybir.dt.bfloat16, name="warm_r")
        nc.vector.memset(wl[:], 0.0)
        nc.vector.memset(wr[:], 0.0)
        for _ in range(N_WARM):
            nc.tensor.matmul(
                pe_tiles[0][TOKENS:, :], wl[:], wr[:],
                start=True, stop=True, tile_position=(0, TOKENS),
                skip_group_check=True,
            )

        # --- inputs to SBUF ---
        bpt_all = consts.tile([P, 4 * NW], mybir.dt.uint16, name="bpt_all")
        xt_s = consts.tile([P, 32 * TOKENS], mybir.dt.bfloat16, name="xt_s")

        nc.sync.dma_start(bpt_all[:, :NW], bpt[0:P, :])
        nc.scalar.dma_start(xt_s[:, :8 * TOKENS], xt[:, :8 * TOKENS])
        for c in range(1, 4):
            nc.sync.dma_start(bpt_all[:, c * NW:(c + 1) * NW],
                              bpt[c * P:(c + 1) * P, :])
        nc.scalar.dma_start(xt_s[:, 8 * TOKENS:], xt[:, 8 * TOKENS:])

        scale_s = consts.tile([P, KCH], mybir.dt.float32, name="scale_s")
        nc.scalar.dma_start(scale_s[:], scale_t[:, :])

        # M2: [128, 64] = [identity_64; identity_64] — the epilogue matmul
        # ybuf_chunk.T @ M2 transposes y AND sums the parity halves.
        m2 = consts.tile([P, TOKENS], mybir.dt.float16, name="m2")
        make_identity(nc, m2[:TOKENS, :])
        make_identity(nc, m2[TOKENS:, :])

        ybuf = out_pool.tile([P, OPC], mybir.dt.float16, name="ybuf")
        out_s = out_pool.tile([P, KCH * TOKENS], mybir.dt.float32, name="out_s")

        def plane_mm(plane_u16, j, c, og, wcol0):
            """plane_u16: uint16 tile; wcol0: u16 col where chunk c starts."""
            m = c * 8 + j
            half = j % 2
            base = half * TOKENS
            tiles = po_tiles if half else pe_tiles
            w = OG_SIZES[og]
            s0 = wcol0 + (OG_STARTS[og] // 2)
            rhs = plane_u16[:, s0:s0 + w // 2].bitcast(mybir.dt.float8e4)
            nc.tensor.matmul(
                tiles[og][base:base + TOKENS, :],
                xt_s[:, m * TOKENS:(m + 1) * TOKENS],
                rhs,
                start=(c == 0 and j == half),
                stop=(c == 3 and j == 6 + half),
                tile_position=(0, base),
            )

        def epilogue_og(og):
            w = OG_SIZES[og]
            s0, s1 = OG_STARTS[og], OG_STARTS[og] + w
            # PSUM -> SBUF fp16; even half on DVE, odd half on ScalarE
            nc.vector.tensor_copy(ybuf[:TOKENS, s0:s1],
                                  pe_tiles[og][:TOKENS, :])
            nc.scalar.copy(ybuf[TOKENS:, s0:s1], po_tiles[og][TOKENS:, :])
            ks = list(OG_KS[og])
            pairs = [ks[i:i + 2] for i in range(0, len(ks), 2)]
            for pair in pairs:
                # [128,128].T @ [128,64] per chunk: transpose to [o, t] and
                # sum the parity halves via stacked identities; two chunks
                # share one PSUM tile so one DVE op scales both
                pt = psum_t.tile([P, 2 * TOKENS], mybir.dt.float32,
                                 name="psum_t")
                for i, k in enumerate(pair):
                    nc.tensor.matmul(
                        pt[:, i * TOKENS:(i + 1) * TOKENS],
                        ybuf[:, k * P:(k + 1) * P], m2[:, :],
                        start=True, stop=True,
                    )
                k0, n = pair[0], len(pair)
                nc.vector.tensor_tensor(
                    out_s[:, k0 * TOKENS:(k0 + n) * TOKENS].rearrange(
                        "p (n t) -> p n t", n=n),
                    pt[:, :n * TOKENS].rearrange("p (n t) -> p n t", n=n),
                    scale_s[:, k0:k0 + n, None].to_broadcast((P, n, TOKENS)),
                    mybir.AluOpType.mult,
                )

        def extract(j, src, plane_t, tmp_t, lo, hi):
            """Plane j of byte-pair words src[:, lo:hi) -> fp8 +-1 patterns.

            j0: one fused op (w & 0x8080) ^ 0xB8B8; j>=1: two ops via
            t = (w << j) & 0x8080 then t ^ 0xB8B8.
            """
            if j == 0:
                nc.vector.tensor_scalar(
                    plane_t[:, lo:hi], src[:, lo:hi], 0x8080, 0xB8B8,
                    mybir.AluOpType.bitwise_and, mybir.AluOpType.bitwise_xor,
                )
            else:
                nc.vector.tensor_scalar(
                    tmp_t[:, lo:hi], src[:, lo:hi], j, 0x8080,
                    mybir.AluOpType.logical_shift_left,
                    mybir.AluOpType.bitwise_and,
                )
                nc.vector.tensor_scalar(
                    plane_t[:, lo:hi], tmp_t[:, lo:hi], 0xB8B8, None,
                    mybir.AluOpType.bitwise_xor,
                )

        # --- unpack + matmul rounds, c-major; c2+c3 extracted together ---
        for cr in range(3):          # rounds: c0, c1, c2+c3
            wdt = NW if cr < 2 else 2 * NW
            src = bpt_all[:, cr * NW:cr * NW + wdt]
            planes = []
            for j in range(8):
                pt_ = plane_pool.tile([P, wdt], mybir.dt.uint16, name="pl")
                ut_ = (upool.tile([P, wdt], mybir.dt.uint16, name="u")
                       if j else None)
                if cr == 0 and j <= 1:
                    # split so og0's column range unlocks first
                    extract(j, src, pt_, ut_, 0, 256)
                    extract(j, src, pt_, ut_, 256, NW)
                else:
                    extract(j, src, pt_, ut_, 0, wdt)
                planes.append(pt_)

            if cr < 2:
                for j in range(8):
                    for og in range(3):
                        plane_mm(planes[j], j, cr, og, 0)
            else:
                for j in range(8):
                    for og in range(3):
                        plane_mm(planes[j], j, 2, og, 0)
                for og in range(3):
                    for j in range(8):
                        plane_mm(planes[j], j, 3, og, NW)
                    epilogue_og(og)

        # output DMA chunked per og so early chunks overlap the remaining
        # epilogue work
        nc.sync.dma_start(yt[:, :4 * TOKENS], out_s[:, :4 * TOKENS])
        nc.sync.dma_start(yt[:, 4 * TOKENS:8 * TOKENS],
                          out_s[:, 4 * TOKENS:8 * TOKENS])
        nc.sync.dma_start(yt[:, 8 * TOKENS:], out_s[:, 8 * TOKENS:])

    nc.compile()
    return nc


def _prep_inputs(x, bp, scale):
    """Host-side re-layout of the full inputs into 8 per-core input maps."""
    x = np.asarray(x, dtype=np.float32)
    bp = np.asarray(bp)
    scale = np.asarray(scale, dtype=np.float32)

    # packed bytes, transposed to [g, o_padded], then byte-paired along o
    bpm = np.zeros((G, OUT_PAD), dtype=np.uint8)
    bpm[:, :OUT] = bp.astype(np.uint8).reshape(OUT, G).T
    bpm16 = bpm.reshape(G, OUT_PAD // 2, 2)
    bpw = (bpm16[:, :, 1].astype(np.uint16) << 8) | bpm16[:, :, 0]

    # xt = x.T with rows permuted to (c, j, g%128) order, split into
    # 128-row blocks laid out along the free dim: xt_dev[p, m*64 + t]
    # with m = c*8 + j.
    xT = x.T.astype(np.float32)                     # [IN, TOKENS]
    xr = xT.reshape(G, 8, TOKENS).reshape(4, P, 8, TOKENS)
    xr = xr.transpose(0, 2, 1, 3)                   # [c, j, p, t]
    xt_dev = np.ascontiguousarray(
        xr.reshape(32, P, TOKENS).transpose(1, 0, 2).reshape(P, 32 * TOKENS)
    ).astype(ml_dtypes.bfloat16)

    scale_pad = np.zeros(OUT_PAD, dtype=np.float32)
    scale_pad[:OUT] = scale.reshape(-1)

    in_maps = []
    for cid in range(NCORES):
        osl = slice(cid * OPC // 2, (cid + 1) * OPC // 2)
        sl = slice(cid * OPC, (cid + 1) * OPC)
        in_maps.append({
            "bpt": np.ascontiguousarray(bpw[:, osl]),
            "xt": xt_dev,
            "scale_t": np.ascontiguousarray(
                scale_pad[sl].reshape(KCH, P).T),
        })
    return in_maps


def _assemble(results):
    """results: per-core {'yt': [128, 11*64]} -> full [64, OUT] fp32."""
    parts = []
    for cid in range(NCORES):
        a = np.asarray(results[cid]["yt"], dtype=np.float32)
        parts.append(a.reshape(P, KCH, TOKENS).transpose(1, 0, 2).reshape(OPC, TOKENS))
    full = np.concatenate(parts, axis=0)[:OUT]      # [OUT, TOKENS]
    return np.ascontiguousarray(full.T)             # [TOKENS, OUT]


def kernel(x, bp, scale, _trace=False):
    from concourse import bass_utils

    if "nc" not in _CACHE:
        _CACHE["nc"] = _build_bass()
    nc = _CACHE["nc"]

    in_maps = _prep_inputs(x, bp, scale)
    res = bass_utils.run_bass_kernel_spmd(
        nc, in_maps, core_ids=list(range(NCORES)), trace=_trace,
    )
    _CACHE["last_result"] = res
    return _assemble(res.results)
